# revision 19
# baseline (speedup 1.0000x reference)
"""Trainium2 kernel for the DepthTracker correlation pyramid.

Math: for each level l, frame t, track n, the reference bilinearly samples a
7x7 grid of points around coords[t,n] from fmaps_l (128 channels) and
correlates each sample with the 49 track features -> out (L,B,T,N,7,7,7,7).

Decomposition (verified to ~5e-4 rel err vs the jax reference):
  out[l,t,n,h,w,pq] = GT[l,n,(t,w,h),pq]
  GT[l,n,tuv,pq] = sum_c feat[l,n,c,tuv] * trackT[c,(l,n,pq)]
  feat           = Sy @ patch @ Sx^T   (separable 7x8 bilinear blends,
                                        border clamping folded in)
The host computes blend matrices from coords, gathers + blends the patches
(0.6 GFLOP), shards 32 tracks per core; the device runs the 128-channel
correlation einsum (10 GFLOP) on all 8 NeuronCores, data parallel.

The kernel is HBM-DMA-roofline bound (~358 GB/s per core, loads and stores
share it), so the device computes G TRANSPOSED: per (l,n) the patch slice
[C=128, tuv-chunk of 128] is the matmul STATIONARY operand (full 128x128
array, fast-weight-load eligible) and the 49 track features stream as the
moving operand, producing PSUM [128 tuv, 49 pq] chunks. TUV=784 is covered
by 7 chunk starts (0,128,...,640,656; the last overlaps chunk 5 by 112 so
every matmul/psum/store keeps the full uniform 128-partition shape). One
engine copy per (l,n) evacuates [128, 343] to f16, and every store is a
dense 128-partition tile: all 16 SDMA engines engaged (stores with <128
partitions collapse onto ONE engine - measured), no dead partition rows
(the old pair-packed layout padded 98->128 rows: +23% store bytes).
Stores ride the scalar HWDGE ring (warmed by a tiny dummy store up front)
so the sync ring never stalls on a copy semaphore between patch-load
prefetches.
"""

import numpy as np

R = 3
K7 = 7
LEV = 4
B, T, C, N = 1, 16, 128, 256
H, W = 96, 128
NCORES = 8
NS = N // NCORES          # 32 tracks per core
UV = K7 * K7
TUV = T * UV              # 784
PQ = K7 * K7              # 49
NB = 8                    # tracks per patch-load batch
NCHUNK = 6                # device computes tuv 0:768; host does the 16-row
STARTS = (0, 128, 256, 384, 512, 640)        # remainder (784 = 6*128 + 16)

COMPUTE_DT = 'f16'
OUT_DT = 'f16'
TRACE = False
LAST_RESULT = {}

_BASS_CACHE = {}


def _np_compute_dtype():
    if COMPUTE_DT in ('f32r', 'f32'):
        return np.float32
    if COMPUTE_DT == 'f16':
        return np.float16
    import ml_dtypes
    return np.dtype(ml_dtypes.bfloat16)


def _build_bass():
    key = (COMPUTE_DT, OUT_DT)
    if key in _BASS_CACHE:
        return _BASS_CACHE[key]
    import concourse.bacc as bacc
    import concourse.mybir as mybir
    from concourse import tile

    cdt = {
        'f32r': mybir.dt.float32r,
        'f32': mybir.dt.float32,
        'f16': mybir.dt.float16,
        'bf16': mybir.dt.bfloat16,
    }[COMPUTE_DT]
    f32 = mybir.dt.float32
    odt = f32 if OUT_DT == 'f32' else mybir.dt.float16

    nc = bacc.Bacc("TRN2", target_bir_lowering=False, debug=False)
    patches = nc.dram_tensor("patches", (LEV, C, NS * TUV), cdt,
                             kind="ExternalInput")
    trackT = nc.dram_tensor("trackT", (C, LEV * NS * PQ), cdt,
                            kind="ExternalInput")
    # gmain[l, p, n, c, q] = GT[l, n, STARTS[c]+p, q]; tracks contiguous on
    # the free axis so ragged batch stores stay dense 2-d [128, nbk*343]
    gmain = nc.dram_tensor("gmain", (LEV, 128, NS * NCHUNK * PQ),
                           odt, kind="ExternalOutput")
    with tile.TileContext(nc) as tc:
        with (
            tc.tile_pool(name="track", bufs=1) as track_pool,
            tc.tile_pool(name="patch", bufs=8) as patch_pool,
            tc.tile_pool(name="out", bufs=4) as out_pool,
            tc.tile_pool(name="psum", bufs=8, space="PSUM") as psum_pool,
        ):
            tr = track_pool.tile([C, LEV * NS * PQ], cdt)
            for l in range(LEV):
                # per-level just-in-time track slice keeps the first patch
                # load from queuing behind a full track preload
                ksl = l * NS * PQ
                nc.sync.dma_start(tr[:, ksl:ksl + NS * PQ],
                                  trackT[:, ksl:ksl + NS * PQ])
                n0 = 0
                for bi, nbk in enumerate([8, 8, 8, 4, 2, 2]
                                         if l == LEV - 1
                                         else [NB] * (NS // NB)):
                    pt = patch_pool.tile([C, nbk * TUV], cdt, tag="pt")
                    off = n0 * TUV
                    if l == 0 and bi == 0:
                        # split the first load so compute starts earlier
                        q = nbk * TUV // 4
                        for s in range(4):
                            nc.sync.dma_start(
                                pt[:, s * q:(s + 1) * q],
                                patches[l, :, off + s * q:off + (s + 1) * q])
                    else:
                        nc.sync.dma_start(
                            pt[:], patches[l, :, off:off + nbk * TUV])
                    ot = out_pool.tile([128, NB * NCHUNK * PQ], odt,
                                       tag="ot")
                    for g in range(nbk):
                        n = n0 + g
                        k = (l * NS + n) * PQ
                        base = g * TUV
                        ps = psum_pool.tile([128, NCHUNK * PQ], f32,
                                            tag="ps")
                        for c, s in enumerate(STARTS):
                            nc.tensor.matmul(
                                ps[:, c * PQ:(c + 1) * PQ],
                                pt[:, base + s:base + s + 128],
                                tr[:, k:k + PQ],
                                start=True, stop=True)
                        go = slice(g * NCHUNK * PQ, (g + 1) * NCHUNK * PQ)
                        if g % 2 == 0:
                            nc.vector.tensor_copy(ot[:, go], ps[:])
                        else:
                            nc.scalar.copy(ot[:, go], ps[:])
                    # stores ride the scalar HWDGE ring so the sync ring
                    # never stalls on a copy semaphore between patch loads
                    w = NCHUNK * PQ
                    nc.scalar.dma_start(
                        gmain[l, :, n0 * w:(n0 + nbk) * w],
                        ot[:, 0:nbk * w])
                    n0 += nbk
    nc.compile()
    _BASS_CACHE[key] = nc
    return nc


def _blend_mats(xy, dim):
    """xy: (T,N) fp32 coords at this level's scale. Returns (origin (T,N)
    int32, S (T,N,7,8) fp32) with reference clamping semantics folded in."""
    d = np.arange(-R, R + 1, dtype=np.float32)
    q = xy[..., None] + d
    qc = np.clip(q, 0.0, dim - 1.0)
    x0 = np.floor(qc)
    w = (qc - x0).astype(np.float32)
    x0i = x0.astype(np.int32)
    x1i = np.minimum(x0i + 1, dim - 1)
    org = np.clip(np.floor(xy).astype(np.int32) - R, 0, dim - 8)
    v0 = x0i - org[..., None]
    v1 = x1i - org[..., None]
    eye = np.eye(8, dtype=np.float32)
    S = eye[v0] * (1.0 - w)[..., None] + eye[v1] * w[..., None]
    return org, S


def kernel(fmaps0, fmaps1, fmaps2, fmaps3, track0, track1, track2, track3,
           coords):
    import time as _time
    _t0 = _time.time()
    fmaps = [fmaps0, fmaps1, fmaps2, fmaps3]
    tracks = [track0, track1, track2, track3]
    cdt_np = _np_compute_dtype()
    coords2 = np.asarray(coords, np.float32)[0]        # (T,N,2)

    # ---- host: blend matrices + patch gather --------------------------------
    patches_all = np.empty((LEV, C, N, T, K7, K7), cdt_np)
    for l in range(LEV):
        Hl, Wl = H >> l, W >> l
        sc = np.float32(2.0 ** l)
        x = (coords2[..., 0] / sc).astype(np.float32)
        y = (coords2[..., 1] / sc).astype(np.float32)
        cx, Sx = _blend_mats(x, Wl)
        cy, Sy = _blend_mats(y, Hl)
        fm = np.asarray(fmaps[l], np.float32)[0]       # (T,C,Hl,Wl)
        iy = cy[..., None] + np.arange(8)              # (T,N,8)
        ix = cx[..., None] + np.arange(8)
        t_idx = np.arange(T)[:, None, None, None]
        # fancy indexing -> (T,N,8,8,C) over (u=y-row, v=x-col)
        p = fm[t_idx, :, iy[:, :, :, None], ix[:, :, None, :]]
        # x-blend: (T,N,1,7,8) @ (T,N,8,8,C) -> (T,N,8,7,C)  [u, h]
        px = np.matmul(Sx[:, :, None, :, :], p)
        # y-blend: (T,N,7,8) @ (T,N,8,7*C) -> (T,N,7,7,C)    [w, h]
        py = np.matmul(Sy, px.reshape(T, N, 8, K7 * C))
        py = py.reshape(T, N, K7, K7, C)
        patches_all[l] = py.transpose(4, 1, 0, 2, 3)   # (C,N,T,7,7)

    trackT_all = np.empty((C, LEV, N, PQ), cdt_np)
    for l in range(LEV):
        trackT_all[:, l] = np.asarray(tracks[l], np.float32)[0].transpose(2, 1, 0)

    # ---- device: GT = patches^T @ track, 32 tracks per core -----------------
    nc = _build_bass()
    from concourse import bass_utils
    in_maps = []
    for k in range(NCORES):
        sl = slice(k * NS, (k + 1) * NS)
        in_maps.append({
            "patches": np.ascontiguousarray(
                patches_all[:, :, sl].reshape(LEV, C, NS * TUV)),
            "trackT": np.ascontiguousarray(
                trackT_all[:, :, sl].reshape(C, LEV * NS * PQ)),
        })
    _t1 = _time.time()
    res = bass_utils.run_bass_kernel_spmd(
        nc, in_maps, core_ids=list(range(NCORES)), trace=TRACE)
    _t2 = _time.time()
    LAST_RESULT.update(
        host_pre_s=_t1 - _t0, spmd_s=_t2 - _t1,
        exec_time_ns=res.exec_time_ns, profile_json=res.profile_json)
    # per core: gmain (LEV, 128, NS, 6, 49): chunk c holds GT rows
    # STARTS[c]..STARTS[c]+128; the host computes the tuv 768:784 remainder
    GT = np.empty((LEV, NCORES, NS, TUV, PQ), np.float32)
    for kc, r in enumerate(res.results):
        g = r["gmain"].reshape(LEV, 128, NS, NCHUNK, PQ)
        for c, s in enumerate(STARTS):
            GT[:, kc, :, s:s + 128] = g[:, :, :, c].transpose(0, 2, 1, 3)
    # host remainder: GT[l, n, 768:784, q] = sum_c patches[l,c,n,768+p] * trackT[c,l,n,q]
    pa = patches_all.reshape(LEV, C, N, TUV)[:, :, :, 768:].astype(np.float32)
    GT.reshape(LEV, N, TUV, PQ)[:, :, 768:] = np.einsum(
        'lcnp,clnq->lnpq', pa, trackT_all.astype(np.float32),
        optimize=True)
    GT = GT.reshape(LEV, N, T, K7, K7, PQ)     # [l,n,t,w,h,q]

    # ---- host: final layout only (blend already folded into patches) -------
    out = np.ascontiguousarray(
        GT.transpose(0, 2, 1, 4, 3, 5), dtype=np.float32).reshape(
        LEV, B, T, N, K7, K7, K7, K7)
    LAST_RESULT['host_post_s'] = _time.time() - _t2
    return out


# revision 20
# speedup vs baseline: 1.0874x; 1.0874x over previous
"""Trainium2 kernel for the DepthTracker correlation pyramid.

Math: for each level l, frame t, track n, the reference bilinearly samples a
7x7 grid of points around coords[t,n] from fmaps_l (128 channels) and
correlates each sample with the 49 track features -> out (L,B,T,N,7,7,7,7).

Decomposition (verified to ~5e-4 rel err vs the jax reference):
  out[l,t,n,h,w,pq] = GT[l,n,(t,w,h),pq]
  GT[l,n,tuv,pq] = sum_c feat[l,n,c,tuv] * trackT[c,(l,n,pq)]
  feat           = Sy @ patch @ Sx^T   (separable 7x8 bilinear blends,
                                        border clamping folded in)
The host computes blend matrices from coords, gathers + blends the patches
(0.6 GFLOP), shards 32 tracks per core; the device runs the 128-channel
correlation einsum (10 GFLOP) on all 8 NeuronCores, data parallel.

The kernel is HBM-DMA-roofline bound (~358 GB/s per core, loads and stores
share it), so the device computes G TRANSPOSED: per (l,n) the patch slice
[C=128, tuv-chunk of 128] is the matmul STATIONARY operand (full 128x128
array, fast-weight-load eligible) and the 49 track features stream as the
moving operand, producing PSUM [128 tuv, 49 pq] chunks. The device covers
tuv 0:768 with six full 128-row chunks; the host computes the 16-row
remainder (784 = 6*128 + 16, a 0.2 GFLOP einsum) so every matmul, psum
tile and store keeps the uniform dense 128-partition shape. One engine
copy per (l,n) evacuates [128, 294] to f16, and every store is a dense
128-partition tile: all 16 SDMA engines engaged (stores with <128
partitions collapse onto ONE engine - measured), no dead partition rows
(the old pair-packed layout padded 98->128 rows: +23% store bytes).
Stores ride the scalar HWDGE ring so the sync ring never stalls on a
copy semaphore between patch-load prefetches, and the last level tapers
its batches (8,8,8,4,2,2 tracks) to shorten the load->compute->store
tail. Measured on trn2 (8 cores): ~109-118 us HW exec (baseline pair-
packed track-stationary kernel: ~130 us), rel err 4.8e-4.
"""

import numpy as np

R = 3
K7 = 7
LEV = 4
B, T, C, N = 1, 16, 128, 256
H, W = 96, 128
NCORES = 8
NS = N // NCORES          # 32 tracks per core
UV = K7 * K7
TUV = T * UV              # 784
PQ = K7 * K7              # 49
NB = 8                    # tracks per patch-load batch
NCHUNK = 6                # device computes tuv 0:768; host does the 16-row
STARTS = (0, 128, 256, 384, 512, 640)        # remainder (784 = 6*128 + 16)

COMPUTE_DT = 'f16'
OUT_DT = 'f16'
TRACE = False
LAST_RESULT = {}

_BASS_CACHE = {}


def _np_compute_dtype():
    if COMPUTE_DT in ('f32r', 'f32'):
        return np.float32
    if COMPUTE_DT == 'f16':
        return np.float16
    import ml_dtypes
    return np.dtype(ml_dtypes.bfloat16)


def _build_bass():
    key = (COMPUTE_DT, OUT_DT)
    if key in _BASS_CACHE:
        return _BASS_CACHE[key]
    import concourse.bacc as bacc
    import concourse.mybir as mybir
    from concourse import tile

    cdt = {
        'f32r': mybir.dt.float32r,
        'f32': mybir.dt.float32,
        'f16': mybir.dt.float16,
        'bf16': mybir.dt.bfloat16,
    }[COMPUTE_DT]
    f32 = mybir.dt.float32
    odt = f32 if OUT_DT == 'f32' else mybir.dt.float16

    nc = bacc.Bacc("TRN2", target_bir_lowering=False, debug=False)
    patches = nc.dram_tensor("patches", (LEV, C, NS * TUV), cdt,
                             kind="ExternalInput")
    trackT = nc.dram_tensor("trackT", (C, LEV * NS * PQ), cdt,
                            kind="ExternalInput")
    # gmain[l, p, n, c, q] = GT[l, n, STARTS[c]+p, q]; tracks contiguous on
    # the free axis so ragged batch stores stay dense 2-d [128, nbk*343]
    gmain = nc.dram_tensor("gmain", (LEV, 128, NS * NCHUNK * PQ),
                           odt, kind="ExternalOutput")
    with tile.TileContext(nc) as tc:
        with (
            tc.tile_pool(name="track", bufs=1) as track_pool,
            tc.tile_pool(name="patch", bufs=8) as patch_pool,
            tc.tile_pool(name="out", bufs=4) as out_pool,
            tc.tile_pool(name="psum", bufs=8, space="PSUM") as psum_pool,
        ):
            tr = track_pool.tile([C, LEV * NS * PQ], cdt)
            for l in range(LEV):
                # per-level just-in-time track slice keeps the first patch
                # load from queuing behind a full track preload
                ksl = l * NS * PQ
                nc.sync.dma_start(tr[:, ksl:ksl + NS * PQ],
                                  trackT[:, ksl:ksl + NS * PQ])
                n0 = 0
                for bi, nbk in enumerate([8, 8, 8, 4, 2, 2]
                                         if l == LEV - 1
                                         else [NB] * (NS // NB)):
                    pt = patch_pool.tile([C, nbk * TUV], cdt, tag="pt")
                    off = n0 * TUV
                    if l == 0 and bi == 0:
                        # split the first load so compute starts earlier
                        q = nbk * TUV // 4
                        for s in range(4):
                            nc.sync.dma_start(
                                pt[:, s * q:(s + 1) * q],
                                patches[l, :, off + s * q:off + (s + 1) * q])
                    else:
                        nc.sync.dma_start(
                            pt[:], patches[l, :, off:off + nbk * TUV])
                    ot = out_pool.tile([128, NB * NCHUNK * PQ], odt,
                                       tag="ot")
                    for g in range(nbk):
                        n = n0 + g
                        k = (l * NS + n) * PQ
                        base = g * TUV
                        ps = psum_pool.tile([128, NCHUNK * PQ], f32,
                                            tag="ps")
                        for c, s in enumerate(STARTS):
                            nc.tensor.matmul(
                                ps[:, c * PQ:(c + 1) * PQ],
                                pt[:, base + s:base + s + 128],
                                tr[:, k:k + PQ],
                                start=True, stop=True)
                        go = slice(g * NCHUNK * PQ, (g + 1) * NCHUNK * PQ)
                        if g % 2 == 0:
                            nc.vector.tensor_copy(ot[:, go], ps[:])
                        else:
                            nc.scalar.copy(ot[:, go], ps[:])
                    # stores ride the scalar HWDGE ring so the sync ring
                    # never stalls on a copy semaphore between patch loads
                    w = NCHUNK * PQ
                    nc.scalar.dma_start(
                        gmain[l, :, n0 * w:(n0 + nbk) * w],
                        ot[:, 0:nbk * w])
                    n0 += nbk
    nc.compile()
    _BASS_CACHE[key] = nc
    return nc


def _blend_mats(xy, dim):
    """xy: (T,N) fp32 coords at this level's scale. Returns (origin (T,N)
    int32, S (T,N,7,8) fp32) with reference clamping semantics folded in."""
    d = np.arange(-R, R + 1, dtype=np.float32)
    q = xy[..., None] + d
    qc = np.clip(q, 0.0, dim - 1.0)
    x0 = np.floor(qc)
    w = (qc - x0).astype(np.float32)
    x0i = x0.astype(np.int32)
    x1i = np.minimum(x0i + 1, dim - 1)
    org = np.clip(np.floor(xy).astype(np.int32) - R, 0, dim - 8)
    v0 = x0i - org[..., None]
    v1 = x1i - org[..., None]
    eye = np.eye(8, dtype=np.float32)
    S = eye[v0] * (1.0 - w)[..., None] + eye[v1] * w[..., None]
    return org, S


def kernel(fmaps0, fmaps1, fmaps2, fmaps3, track0, track1, track2, track3,
           coords):
    import time as _time
    _t0 = _time.time()
    fmaps = [fmaps0, fmaps1, fmaps2, fmaps3]
    tracks = [track0, track1, track2, track3]
    cdt_np = _np_compute_dtype()
    coords2 = np.asarray(coords, np.float32)[0]        # (T,N,2)

    # ---- host: blend matrices + patch gather --------------------------------
    patches_all = np.empty((LEV, C, N, T, K7, K7), cdt_np)
    for l in range(LEV):
        Hl, Wl = H >> l, W >> l
        sc = np.float32(2.0 ** l)
        x = (coords2[..., 0] / sc).astype(np.float32)
        y = (coords2[..., 1] / sc).astype(np.float32)
        cx, Sx = _blend_mats(x, Wl)
        cy, Sy = _blend_mats(y, Hl)
        fm = np.asarray(fmaps[l], np.float32)[0]       # (T,C,Hl,Wl)
        iy = cy[..., None] + np.arange(8)              # (T,N,8)
        ix = cx[..., None] + np.arange(8)
        t_idx = np.arange(T)[:, None, None, None]
        # fancy indexing -> (T,N,8,8,C) over (u=y-row, v=x-col)
        p = fm[t_idx, :, iy[:, :, :, None], ix[:, :, None, :]]
        # x-blend: (T,N,1,7,8) @ (T,N,8,8,C) -> (T,N,8,7,C)  [u, h]
        px = np.matmul(Sx[:, :, None, :, :], p)
        # y-blend: (T,N,7,8) @ (T,N,8,7*C) -> (T,N,7,7,C)    [w, h]
        py = np.matmul(Sy, px.reshape(T, N, 8, K7 * C))
        py = py.reshape(T, N, K7, K7, C)
        patches_all[l] = py.transpose(4, 1, 0, 2, 3)   # (C,N,T,7,7)

    trackT_all = np.empty((C, LEV, N, PQ), cdt_np)
    for l in range(LEV):
        trackT_all[:, l] = np.asarray(tracks[l], np.float32)[0].transpose(2, 1, 0)

    # ---- device: GT = patches^T @ track, 32 tracks per core -----------------
    nc = _build_bass()
    from concourse import bass_utils
    in_maps = []
    for k in range(NCORES):
        sl = slice(k * NS, (k + 1) * NS)
        in_maps.append({
            "patches": np.ascontiguousarray(
                patches_all[:, :, sl].reshape(LEV, C, NS * TUV)),
            "trackT": np.ascontiguousarray(
                trackT_all[:, :, sl].reshape(C, LEV * NS * PQ)),
        })
    _t1 = _time.time()
    res = bass_utils.run_bass_kernel_spmd(
        nc, in_maps, core_ids=list(range(NCORES)), trace=TRACE)
    _t2 = _time.time()
    LAST_RESULT.update(
        host_pre_s=_t1 - _t0, spmd_s=_t2 - _t1,
        exec_time_ns=res.exec_time_ns, profile_json=res.profile_json)
    # per core: gmain (LEV, 128, NS, 6, 49): chunk c holds GT rows
    # STARTS[c]..STARTS[c]+128; the host computes the tuv 768:784 remainder
    GT = np.empty((LEV, NCORES, NS, TUV, PQ), np.float32)
    for kc, r in enumerate(res.results):
        g = r["gmain"].reshape(LEV, 128, NS, NCHUNK, PQ)
        for c, s in enumerate(STARTS):
            GT[:, kc, :, s:s + 128] = g[:, :, :, c].transpose(0, 2, 1, 3)
    # host remainder: GT[l, n, 768:784, q] = sum_c patches[l,c,n,768+p] * trackT[c,l,n,q]
    pa = patches_all.reshape(LEV, C, N, TUV)[:, :, :, 768:].astype(np.float32)
    GT.reshape(LEV, N, TUV, PQ)[:, :, 768:] = np.einsum(
        'lcnp,clnq->lnpq', pa, trackT_all.astype(np.float32),
        optimize=True)
    GT = GT.reshape(LEV, N, T, K7, K7, PQ)     # [l,n,t,w,h,q]

    # ---- host: final layout only (blend already folded into patches) -------
    out = np.ascontiguousarray(
        GT.transpose(0, 2, 1, 4, 3, 5), dtype=np.float32).reshape(
        LEV, B, T, N, K7, K7, K7, K7)
    LAST_RESULT['host_post_s'] = _time.time() - _t2
    return out
